# revision 7
# baseline (speedup 1.0000x reference)
"""Multi-head attention (B=2, S=2048, D=1024, H=16, DH=64) on 8 TRN2 NeuronCores.

Sharding: 8 cores = 2 batches x 4 head-groups. Core c handles batch b = c % 2
and heads [4g, 4g+4) with g = c // 2.  Each core computes, for its batch:
  - per-head q/k projections in transposed layout qT/kT [DH, S]
  - v projection in normal layout [S, DH] (with an appended ones column)
  - scoresT[t, s] = k_h q_h^T / done via PE, exp on ACT
  - softmax denominator for free via the ones column of v in the
    attn @ v matmul (row DH of the PSUM accumulator = column sums)
  - 1/sum via ACT exp(-ln(x)); broadcast across partitions via a PE
    rank-1 matmul (ones [1,128]^T @ rec [1,S])
  - attn (normalized, transposed [t, s]) written to HBM; host transposes back
  - partial output projection out_p[s, o] (host sums the 4 head-group parts)

All host-side work is pure layout (transpose/slice/concat/sum of partials).
"""

import sys

sys.path.insert(0, "/opt/trn_rl_repo")

import numpy as np

import concourse.bass as bass
import concourse.mybir as mybir
import concourse.tile as tile
from concourse import bacc
from concourse import bass_utils

AF = mybir.ActivationFunctionType
F32 = mybir.dt.float32
F32R = mybir.dt.float32r

# Problem geometry (hardcoded per contract)
B, S, D, H, DH = 2, 2048, 1024, 16, 64
NCORES = 8
HPC = H // (NCORES // B)  # 4 heads per core


def _nsplit(width, maxn=512):
    """Split width into <=512 chunks (PE fp32 moving-operand limit)."""
    out = []
    o = 0
    while o < width:
        n = min(maxn, width - o)
        out.append((o, n))
        o += n
    return out


def build_mha(nc, S=S, D=D, HPC=HPC, DH=DH, SH=1024, use_f32r=True):
    """Emit the tile program on nc. Returns None; tensors are declared on nc."""
    P = 128
    fmm = F32R if use_f32r else F32
    E4 = HPC * DH            # concat head dim per core (256)
    VW = HPC * (DH + 1)      # v tile width with ones column per head (260)
    NPAIR = HPC // 2         # head pairs for 128-row projections
    ND = D // P              # contraction chunks
    NT = S // P              # t chunks (key/value rows)
    NSH = S // SH            # s blocks
    SCALE = 1.0 / float(np.sqrt(DH))

    xqT = nc.dram_tensor("xqT", [D, S], fmm, kind="ExternalInput").ap()
    xkT = nc.dram_tensor("xkT", [D, S], fmm, kind="ExternalInput").ap()
    xvT = nc.dram_tensor("xvT", [D, S], fmm, kind="ExternalInput").ap()
    wq = nc.dram_tensor("wq", [D, E4], fmm, kind="ExternalInput").ap()
    wk = nc.dram_tensor("wk", [D, E4], fmm, kind="ExternalInput").ap()
    wv = nc.dram_tensor("wv", [D, E4], fmm, kind="ExternalInput").ap()
    woT = nc.dram_tensor("woT", [E4, D], fmm, kind="ExternalInput").ap()
    out_p = nc.dram_tensor("out_p", [S, D], F32, kind="ExternalOutput").ap()
    attn_t = nc.dram_tensor("attn_t", [HPC, S, S], F32, kind="ExternalOutput").ap()

    with tile.TileContext(nc) as tc:
        with tc.tile_pool(name="pers", bufs=1) as pers:
            # persistent SBUF tensors
            qT = [pers.tile([P, S], fmm, name=f"qT{g}", tag=f"qT{g}")
                  for g in range(NPAIR)]
            kT = [pers.tile([P, S], fmm, name=f"kT{g}", tag=f"kT{g}")
                  for g in range(NPAIR)]
            vt = [pers.tile([P, VW], fmm, name=f"v{t}", tag=f"v{t}")
                  for t in range(NT)]
            wt = [pers.tile([DH, S], fmm, name=f"wt{h}", tag=f"wt{h}")
                  for h in range(HPC)]
            woTs = [pers.tile([DH, D], fmm, name=f"woT{h}", tag=f"woT{h}")
                    for h in range(HPC)]
            ones_row = pers.tile([1, P], fmm, name="ones_row", tag="ones_row")
            onesc = pers.tile([P, P], F32, name="onesc", tag="onesc")

            # Memset can't write f32r; memset an f32 scratch then copy-cast.
            nc.vector.memset(onesc[:, :], 1.0)
            nc.vector.tensor_copy(ones_row[:, :], onesc[0:1, :])
            for t in range(NT):
                # ones column at local index DH of each head's 65-wide slot
                col = vt[t].rearrange("p (h e) -> p h e", e=DH + 1)[:, :, DH]
                nc.vector.tensor_copy(col, onesc[:, 0:HPC])
            for h in range(HPC):
                nc.sync.dma_start(woTs[h][:, :], woT[h * DH:(h + 1) * DH, :])

            # ---------------- phase 1: projections ----------------
            with tc.tile_pool(name="xs", bufs=4) as xs, \
                 tc.tile_pool(name="ws", bufs=1) as ws, \
                 tc.tile_pool(name="pp", bufs=2, space="PSUM") as pp:

                for nm, xT_d, w_d, dst in (("q", xqT, wq, qT), ("k", xkT, wk, kT)):
                    wtiles = []
                    for kd in range(ND):
                        w_t = ws.tile([P, E4], fmm, name=f"w{nm}{kd}", tag=f"w{kd}")
                        nc.sync.dma_start(w_t[:, :], w_d[kd * P:(kd + 1) * P, :])
                        wtiles.append(w_t)
                    for sj in range(S // 512):
                        ps = [pp.tile([P, 512], F32, name=f"p{nm}{g}", tag=f"pq{g}")
                              for g in range(NPAIR)]
                        for kd in range(ND):
                            xt = xs.tile([P, 512], fmm, name=f"x{nm}", tag="x")
                            nc.sync.dma_start(
                                xt[:, :], xT_d[kd * P:(kd + 1) * P, sj * 512:(sj + 1) * 512])
                            for g in range(NPAIR):
                                nc.tensor.matmul(
                                    ps[g][:, :],
                                    wtiles[kd][:, g * 2 * DH:(g + 1) * 2 * DH],
                                    xt[:, :],
                                    start=(kd == 0), stop=(kd == ND - 1))
                        for g in range(NPAIR):
                            nc.vector.tensor_copy(
                                dst[g][:, sj * 512:(sj + 1) * 512], ps[g][:, :])

                # v projection (normal layout)
                vw_tiles = []
                for kd in range(ND):
                    w_t = ws.tile([P, E4], fmm, name=f"wv{kd}", tag=f"w{kd}")
                    nc.sync.dma_start(w_t[:, :], wv[kd * P:(kd + 1) * P, :])
                    vw_tiles.append(w_t)
                for tg in range(S // 512):
                    pv = [pp.tile([P, E4], F32, name=f"pv{i}", tag=f"pv{i}", bufs=1)
                          for i in range(4)]
                    for kd in range(ND):
                        xt = xs.tile([P, 512], fmm, name="xv", tag="x")
                        nc.sync.dma_start(
                            xt[:, :], xvT[kd * P:(kd + 1) * P, tg * 512:(tg + 1) * 512])
                        for i in range(4):
                            nc.tensor.matmul(
                                pv[i][:, :],
                                xt[:, i * P:(i + 1) * P],
                                vw_tiles[kd][:, :],
                                start=(kd == 0), stop=(kd == ND - 1))
                    for i in range(4):
                        t = tg * 4 + i
                        dstv = vt[t].rearrange("p (h e) -> p h e", e=DH + 1)[:, :, 0:DH]
                        nc.vector.tensor_copy(
                            dstv, pv[i].rearrange("p (h e) -> p h e", e=DH))

            # ---------------- phase 2: attention ----------------
            with tc.tile_pool(name="et", bufs=1) as etp, \
                 tc.tile_pool(name="rc", bufs=2) as rcp, \
                 tc.tile_pool(name="sm", bufs=2) as smp, \
                 tc.tile_pool(name="sc", bufs=2, space="PSUM") as scp, \
                 tc.tile_pool(name="pw", bufs=2, space="PSUM") as pwp:

                for h in range(HPC):
                    g, r = divmod(h, 2)
                    qTh = qT[g][r * DH:(r + 1) * DH, :]
                    kTh = kT[g][r * DH:(r + 1) * DH, :]
                    vslot = slice(h * (DH + 1), (h + 1) * (DH + 1))
                    for si in range(NSH):
                        s0 = si * SH
                        pw = pwp.tile([DH + 1, SH], F32, name="pw", tag="pw")
                        ets = []
                        for t in range(NT):
                            sct = scp.tile([P, SH], F32, name="sct", tag="sc")
                            for o, n in _nsplit(SH):
                                nc.tensor.matmul(
                                    sct[:, o:o + n],
                                    kTh[:, t * P:(t + 1) * P],
                                    qTh[:, s0 + o:s0 + o + n],
                                    start=True, stop=True)
                            et = etp.tile([P, SH], fmm, name=f"et{t}", tag=f"et{t}")
                            nc.scalar.activation(et[:, :], sct[:, :], AF.Exp, scale=SCALE)
                            for o, n in _nsplit(SH):
                                nc.tensor.matmul(
                                    pw[:, o:o + n],
                                    vt[t][:, vslot],
                                    et[:, o:o + n],
                                    start=(t == 0), stop=(t == NT - 1))
                            ets.append(et)
                        # reciprocal of column sums (row DH of pw):
                        # ACT copies the sums row out of PSUM (keeps DVE free),
                        # DMA hops partition 64 -> 0, ACT computes exp(-ln(x)),
                        # GPSIMD broadcasts partition 0 to all 128 partitions.
                        sums = smp.tile([P, SH], fmm, name="sums", tag="sums")
                        nc.scalar.copy(sums[DH:DH + 1, :], pw[DH:DH + 1, :])
                        nc.sync.dma_start(sums[0:1, :], sums[DH:DH + 1, :])
                        nc.scalar.activation(sums[0:1, :], sums[0:1, :].bitcast(F32), AF.Ln)
                        nc.scalar.activation(sums[0:1, :], sums[0:1, :].bitcast(F32), AF.Exp,
                                             scale=-1.0)
                        rec = rcp.tile([P, SH], F32, name="rec", tag="rec")
                        nc.gpsimd.partition_broadcast(rec[:, :], sums[0:1, :].bitcast(F32))
                        # weightedT eviction fused with normalization
                        nc.vector.tensor_mul(
                            wt[h][:, s0:s0 + SH], pw[0:DH, :], rec[0:DH, :])
                        # attn normalization + writeout (transposed layout)
                        for t in range(NT):
                            nc.vector.tensor_mul(ets[t][:, :], ets[t][:, :].bitcast(F32), rec[:, :])
                            nc.sync.dma_start(
                                attn_t[h, t * P:(t + 1) * P, s0:s0 + SH], ets[t][:, :].bitcast(F32))

            # ---------------- phase 3: output projection ----------------
            with tc.tile_pool(name="ob", bufs=3) as obp, \
                 tc.tile_pool(name="po", bufs=4, space="PSUM") as pop:
                for sj in range(NT):
                    ob = obp.tile([P, D], F32, name="ob", tag="ob")
                    for o, n in _nsplit(D):
                        po = pop.tile([P, 512], F32, name="po", tag="po")
                        for h in range(HPC):
                            nc.tensor.matmul(
                                po[:, 0:n],
                                wt[h][:, sj * P:(sj + 1) * P],
                                woTs[h][:, o:o + n],
                                start=(h == 0), stop=(h == HPC - 1))
                        nc.scalar.copy(ob[:, o:o + n], po[:, 0:n])
                    nc.sync.dma_start(out_p[sj * P:(sj + 1) * P, :], ob[:, :])


_NC_CACHE = {}


class _MhaBacc(bacc.Bacc):
    """Bacc whose activation-table pass only sees natural_log_exp_and_others.

    The default fixpoint alternates exp_and_others <-> natural_log (one
    ~2.7us ACT_TABLE_LOAD per switch, 17 switches here). Every activation
    this kernel uses (Exp, Ln, Copy) lives in natural_log_exp_and_others,
    so restrict the table list (keeping positions, which are the set ids)
    to force a single load.
    """

    def insert_act_table_loads(self):
        import bass_rust as _bass_rust
        from concourse.hw_specs import get_activation_tables
        has_activation = any(
            isinstance(i, mybir.InstActivation)
            for b in self.main_func.blocks
            for i in b.instructions)
        if not has_activation:
            return
        tables = []
        for name, fns in get_activation_tables(self.m.arch).items():
            tables.append((name, fns if name == "natural_log_exp_and_others"
                           else set()))
        _bass_rust.insert_act_table_loads(self, tables)


def _get_compiled(use_f32r=True):
    key = ("full", use_f32r)
    if key not in _NC_CACHE:
        nc = _MhaBacc("TRN2", target_bir_lowering=False, debug=False,
                      enable_asserts=True, num_devices=NCORES)
        build_mha(nc, use_f32r=use_f32r)
        nc.compile()
        _NC_CACHE[key] = nc
    return _NC_CACHE[key]


def make_in_maps(query, key, value, Wq, Wk, Wv, Wo):
    query = np.asarray(query, dtype=np.float32)
    key = np.asarray(key, dtype=np.float32)
    value = np.asarray(value, dtype=np.float32)
    Wq = np.asarray(Wq, dtype=np.float32)
    Wk = np.asarray(Wk, dtype=np.float32)
    Wv = np.asarray(Wv, dtype=np.float32)
    Wo = np.asarray(Wo, dtype=np.float32)
    in_maps = []
    for c in range(NCORES):
        b, g = c % B, c // B
        hs = list(range(HPC * g, HPC * (g + 1)))
        in_maps.append({
            "xqT": np.ascontiguousarray(query[b].T),
            "xkT": np.ascontiguousarray(key[b].T),
            "xvT": np.ascontiguousarray(value[b].T),
            "wq": np.ascontiguousarray(np.concatenate([Wq[h] for h in hs], axis=1)),
            "wk": np.ascontiguousarray(np.concatenate([Wk[h] for h in hs], axis=1)),
            "wv": np.ascontiguousarray(np.concatenate([Wv[h] for h in hs], axis=1)),
            "woT": np.ascontiguousarray(
                Wo[:, HPC * DH * g:HPC * DH * (g + 1)].T),
        })
    return in_maps


def assemble(results):
    output = np.zeros((B, S, D), np.float32)
    attn = np.empty((H, B, S, S), np.float32)
    for c in range(NCORES):
        b, g = c % B, c // B
        output[b] += results[c]["out_p"]
        at = results[c]["attn_t"]
        for j in range(HPC):
            attn[HPC * g + j, b] = at[j].T
    return output, attn


def kernel(query, key, value, Wq, Wk, Wv, Wo, _trace=False):
    nc = _get_compiled()
    in_maps = make_in_maps(query, key, value, Wq, Wk, Wv, Wo)
    res = bass_utils.run_bass_kernel_spmd(
        nc, in_maps, core_ids=list(range(NCORES)), trace=_trace)
    out = assemble(res.results)
    if _trace:
        return out, res
    return out


# revision 9
# speedup vs baseline: 1.2960x; 1.2960x over previous
"""Multi-head attention (B=2, S=2048, D=1024, H=16, DH=64) on 8 TRN2 NeuronCores.

Sharding: 8 cores = 2 batches x 4 head-groups. Core c handles batch b = c % 2
and heads [4g, 4g+4) with g = c // 2.  Each core computes, for its batch:
  - per-head q/k projections in transposed layout qT/kT [DH, S]
  - v projection in normal layout [S, DH] (with an appended ones column)
  - scoresT[t, s] = k_h q_h^T / done via PE, exp on ACT
  - softmax denominator for free via the ones column of v in the
    attn @ v matmul (row DH of the PSUM accumulator = column sums)
  - 1/sum via ACT exp(-ln(x)); broadcast across partitions via a PE
    rank-1 matmul (ones [1,128]^T @ rec [1,S])
  - attn (normalized, transposed [t, s]) written to HBM; host transposes back
  - partial output projection out_p[s, o] (host sums the 4 head-group parts)

All host-side work is pure layout (transpose/slice/concat/sum of partials).
"""

import sys

sys.path.insert(0, "/opt/trn_rl_repo")

import numpy as np

import concourse.bass as bass
import concourse.mybir as mybir
import concourse.tile as tile
from concourse import bacc
from concourse import bass_utils

AF = mybir.ActivationFunctionType
F32 = mybir.dt.float32
F32R = mybir.dt.float32r

# Problem geometry (hardcoded per contract)
B, S, D, H, DH = 2, 2048, 1024, 16, 64
NCORES = 8
HPC = H // (NCORES // B)  # 4 heads per core


def _nsplit(width, maxn=512):
    """Split width into <=512 chunks (PE fp32 moving-operand limit)."""
    out = []
    o = 0
    while o < width:
        n = min(maxn, width - o)
        out.append((o, n))
        o += n
    return out


def build_mha(nc, S=S, D=D, HPC=HPC, DH=DH, SH=1024, use_f32r=True):
    """Emit the tile program on nc. Returns None; tensors are declared on nc."""
    P = 128
    fmm = F32R if use_f32r else F32
    E4 = HPC * DH            # concat head dim per core (256)
    VW = HPC * (DH + 1)      # v tile width with ones column per head (260)
    NPAIR = HPC // 2         # head pairs for 128-row projections
    ND = D // P              # contraction chunks
    NT = S // P              # t chunks (key/value rows)
    NSH = S // SH            # s blocks
    SCALE = 1.0 / float(np.sqrt(DH))

    xqT = nc.dram_tensor("xqT", [D, S], fmm, kind="ExternalInput").ap()
    xkT = nc.dram_tensor("xkT", [D, S], fmm, kind="ExternalInput").ap()
    xvT = nc.dram_tensor("xvT", [D, S], fmm, kind="ExternalInput").ap()
    wq = nc.dram_tensor("wq", [D, E4], fmm, kind="ExternalInput").ap()
    wk = nc.dram_tensor("wk", [D, E4], fmm, kind="ExternalInput").ap()
    wv = nc.dram_tensor("wv", [D, E4], fmm, kind="ExternalInput").ap()
    woT = nc.dram_tensor("woT", [E4, D], fmm, kind="ExternalInput").ap()
    out_p = nc.dram_tensor("out_p", [S, D], F32, kind="ExternalOutput").ap()
    attn_t = nc.dram_tensor("attn_t", [HPC, S, S], F32, kind="ExternalOutput").ap()

    with tile.TileContext(nc) as tc:
        with tc.tile_pool(name="pers", bufs=1) as pers:
            # persistent SBUF tensors
            qT = [pers.tile([P, S], fmm, name=f"qT{g}", tag=f"qT{g}")
                  for g in range(NPAIR)]
            kT = [pers.tile([P, S], fmm, name=f"kT{g}", tag=f"kT{g}")
                  for g in range(NPAIR)]
            vt = [pers.tile([P, VW], fmm, name=f"v{t}", tag=f"v{t}")
                  for t in range(NT)]
            wt = [pers.tile([DH, S], fmm, name=f"wt{h}", tag=f"wt{h}")
                  for h in range(HPC)]
            woTs = [pers.tile([DH, D], fmm, name=f"woT{h}", tag=f"woT{h}")
                    for h in range(HPC)]
            onesc = pers.tile([P, P], F32, name="onesc", tag="onesc")

            # Memset can't write f32r; memset an f32 scratch then copy-cast.
            nc.vector.memset(onesc[:, :], 1.0)
            for t in range(NT):
                # ones column at local index DH of each head's 65-wide slot
                col = vt[t].rearrange("p (h e) -> p h e", e=DH + 1)[:, :, DH]
                nc.vector.tensor_copy(col, onesc[:, 0:HPC])
            for h in range(HPC):
                nc.sync.dma_start(woTs[h][:, :], woT[h * DH:(h + 1) * DH, :])

            # ---------------- phase 1: projections ----------------
            with tc.tile_pool(name="xs", bufs=6) as xs, \
                 tc.tile_pool(name="ws", bufs=2) as ws, \
                 tc.tile_pool(name="pp", bufs=2, space="PSUM") as pp:

                for nm, xT_d, w_d, dst in (("q", xqT, wq, qT), ("k", xkT, wk, kT)):
                    wtiles = []
                    for kd in range(ND):
                        w_t = ws.tile([P, E4], fmm, name=f"w{nm}{kd}", tag=f"w{kd}")
                        nc.sync.dma_start(w_t[:, :], w_d[kd * P:(kd + 1) * P, :])
                        wtiles.append(w_t)
                    for sj in range(S // 512):
                        ps = [pp.tile([P, 512], F32, name=f"p{nm}{g}", tag=f"pq{g}")
                              for g in range(NPAIR)]
                        for kd in range(ND):
                            xt = xs.tile([P, 512], fmm, name=f"x{nm}", tag="x")
                            nc.sync.dma_start(
                                xt[:, :], xT_d[kd * P:(kd + 1) * P, sj * 512:(sj + 1) * 512])
                            for g in range(NPAIR):
                                nc.tensor.matmul(
                                    ps[g][:, :],
                                    wtiles[kd][:, g * 2 * DH:(g + 1) * 2 * DH],
                                    xt[:, :],
                                    start=(kd == 0), stop=(kd == ND - 1))
                        for g in range(NPAIR):
                            nc.scalar.copy(
                                dst[g][:, sj * 512:(sj + 1) * 512], ps[g][:, :])

                # v projection (normal layout)
                vw_tiles = []
                for kd in range(ND):
                    w_t = ws.tile([P, E4], fmm, name=f"wv{kd}", tag=f"w{kd}")
                    nc.sync.dma_start(w_t[:, :], wv[kd * P:(kd + 1) * P, :])
                    vw_tiles.append(w_t)
                for tg in range(S // 512):
                    pv = [pp.tile([P, E4], F32, name=f"pv{i}", tag=f"pv{i}", bufs=1)
                          for i in range(4)]
                    for kd in range(ND):
                        xt = xs.tile([P, 512], fmm, name="xv", tag="x")
                        nc.sync.dma_start(
                            xt[:, :], xvT[kd * P:(kd + 1) * P, tg * 512:(tg + 1) * 512])
                        for i in range(4):
                            nc.tensor.matmul(
                                pv[i][:, :],
                                xt[:, i * P:(i + 1) * P],
                                vw_tiles[kd][:, :],
                                start=(kd == 0), stop=(kd == ND - 1))
                    for i in range(4):
                        t = tg * 4 + i
                        dstv = vt[t].rearrange("p (h e) -> p h e", e=DH + 1)[:, :, 0:DH]
                        nc.scalar.copy(
                            dstv, pv[i].rearrange("p (h e) -> p h e", e=DH))

            # ---------------- phase 2: attention ----------------
            with tc.tile_pool(name="et", bufs=1) as etp, \
                 tc.tile_pool(name="rc", bufs=2) as rcp, \
                 tc.tile_pool(name="sm", bufs=1) as smp, \
                 tc.tile_pool(name="sc", bufs=3, space="PSUM") as scp, \
                 tc.tile_pool(name="pw", bufs=1, space="PSUM") as pwp:

                for h in range(HPC):
                    g, r = divmod(h, 2)
                    qTh = qT[g][r * DH:(r + 1) * DH, :]
                    kTh = kT[g][r * DH:(r + 1) * DH, :]
                    vslot = slice(h * (DH + 1), (h + 1) * (DH + 1))
                    for si in range(NSH):
                        s0 = si * SH
                        pw = pwp.tile([DH + 1, SH], F32, name="pw", tag="pw")
                        ets = []
                        for t in range(NT):
                            sct = scp.tile([P, SH], F32, name="sct", tag="sc")
                            for o, n in _nsplit(SH):
                                nc.tensor.matmul(
                                    sct[:, o:o + n],
                                    kTh[:, t * P:(t + 1) * P],
                                    qTh[:, s0 + o:s0 + o + n],
                                    start=True, stop=True)
                            et = etp.tile([P, SH], fmm, name=f"et{t}", tag="et", bufs=24)
                            nc.scalar.activation(et[:, :], sct[:, :], AF.Exp, scale=SCALE)
                            for o, n in _nsplit(SH):
                                nc.tensor.matmul(
                                    pw[:, o:o + n],
                                    vt[t][:, vslot],
                                    et[:, o:o + n],
                                    start=(t == 0), stop=(t == NT - 1))
                            ets.append(et)
                        # reciprocal of column sums (row DH of pw):
                        # ACT copies the sums row out of PSUM (keeps DVE free),
                        # DMA hops partition 64 -> 0, ACT computes exp(-ln(x)),
                        # GPSIMD broadcasts partition 0 to all 128 partitions.
                        sums = smp.tile([P, SH], fmm, name="sums", tag="sums")
                        nc.scalar.copy(sums[DH:DH + 1, :], pw[DH:DH + 1, :])
                        nc.sync.dma_start(sums[0:1, :], sums[DH:DH + 1, :])
                        nc.scalar.activation(sums[0:1, :], sums[0:1, :].bitcast(F32), AF.Ln)
                        nc.scalar.activation(sums[0:1, :], sums[0:1, :].bitcast(F32), AF.Exp,
                                             scale=-1.0)
                        rec = rcp.tile([P, SH], F32, name="rec", tag="rec")
                        nc.gpsimd.partition_broadcast(rec[:, :], sums[0:1, :].bitcast(F32))
                        # weightedT eviction fused with normalization
                        nc.vector.tensor_mul(
                            wt[h][:, s0:s0 + SH], pw[0:DH, :], rec[0:DH, :])
                        # attn normalization + writeout (transposed layout)
                        for t in range(NT):
                            nc.vector.tensor_mul(ets[t][:, :], ets[t][:, :].bitcast(F32), rec[:, :])
                            nc.sync.dma_start(
                                attn_t[h, t * P:(t + 1) * P, s0:s0 + SH], ets[t][:, :].bitcast(F32))

            # ---------------- phase 3: output projection ----------------
            with tc.tile_pool(name="ob", bufs=3) as obp, \
                 tc.tile_pool(name="po", bufs=4, space="PSUM") as pop:
                for sj in range(NT):
                    ob = obp.tile([P, D], F32, name="ob", tag="ob")
                    for o, n in _nsplit(D):
                        po = pop.tile([P, 512], F32, name="po", tag="po")
                        for h in range(HPC):
                            nc.tensor.matmul(
                                po[:, 0:n],
                                wt[h][:, sj * P:(sj + 1) * P],
                                woTs[h][:, o:o + n],
                                start=(h == 0), stop=(h == HPC - 1))
                        nc.scalar.copy(ob[:, o:o + n], po[:, 0:n])
                    nc.sync.dma_start(out_p[sj * P:(sj + 1) * P, :], ob[:, :])


_NC_CACHE = {}


class _MhaBacc(bacc.Bacc):
    """Bacc whose activation-table pass only sees natural_log_exp_and_others.

    The default fixpoint alternates exp_and_others <-> natural_log (one
    ~2.7us ACT_TABLE_LOAD per switch, 17 switches here). Every activation
    this kernel uses (Exp, Ln, Copy) lives in natural_log_exp_and_others,
    so restrict the table list (keeping positions, which are the set ids)
    to force a single load.
    """

    def insert_act_table_loads(self):
        import bass_rust as _bass_rust
        from concourse.hw_specs import get_activation_tables
        has_activation = any(
            isinstance(i, mybir.InstActivation)
            for b in self.main_func.blocks
            for i in b.instructions)
        if not has_activation:
            return
        tables = []
        for name, fns in get_activation_tables(self.m.arch).items():
            tables.append((name, fns if name == "natural_log_exp_and_others"
                           else set()))
        _bass_rust.insert_act_table_loads(self, tables)


def _get_compiled(use_f32r=True):
    key = ("full", use_f32r)
    if key not in _NC_CACHE:
        nc = _MhaBacc("TRN2", target_bir_lowering=False, debug=False,
                      enable_asserts=True, num_devices=NCORES)
        build_mha(nc, use_f32r=use_f32r)
        nc.compile()
        _NC_CACHE[key] = nc
    return _NC_CACHE[key]


def make_in_maps(query, key, value, Wq, Wk, Wv, Wo):
    query = np.asarray(query, dtype=np.float32)
    key = np.asarray(key, dtype=np.float32)
    value = np.asarray(value, dtype=np.float32)
    Wq = np.asarray(Wq, dtype=np.float32)
    Wk = np.asarray(Wk, dtype=np.float32)
    Wv = np.asarray(Wv, dtype=np.float32)
    Wo = np.asarray(Wo, dtype=np.float32)
    in_maps = []
    for c in range(NCORES):
        b, g = c % B, c // B
        hs = list(range(HPC * g, HPC * (g + 1)))
        in_maps.append({
            "xqT": np.ascontiguousarray(query[b].T),
            "xkT": np.ascontiguousarray(key[b].T),
            "xvT": np.ascontiguousarray(value[b].T),
            "wq": np.ascontiguousarray(np.concatenate([Wq[h] for h in hs], axis=1)),
            "wk": np.ascontiguousarray(np.concatenate([Wk[h] for h in hs], axis=1)),
            "wv": np.ascontiguousarray(np.concatenate([Wv[h] for h in hs], axis=1)),
            "woT": np.ascontiguousarray(
                Wo[:, HPC * DH * g:HPC * DH * (g + 1)].T),
        })
    return in_maps


def assemble(results):
    output = np.zeros((B, S, D), np.float32)
    attn = np.empty((H, B, S, S), np.float32)
    for c in range(NCORES):
        b, g = c % B, c // B
        output[b] += results[c]["out_p"]
        at = results[c]["attn_t"]
        for j in range(HPC):
            attn[HPC * g + j, b] = at[j].T
    return output, attn


def kernel(query, key, value, Wq, Wk, Wv, Wo, _trace=False):
    nc = _get_compiled()
    in_maps = make_in_maps(query, key, value, Wq, Wk, Wv, Wo)
    res = bass_utils.run_bass_kernel_spmd(
        nc, in_maps, core_ids=list(range(NCORES)), trace=_trace)
    out = assemble(res.results)
    if _trace:
        return out, res
    return out
